# revision 30
# baseline (speedup 1.0000x reference)
"""MLA attention Trainium2 kernel: nn_MultiHeadLatentAttention_31722628448847.

Full computation (B=1, T=2048, C=2048, H=16, G=4, Dl=32):
  q  = x @ Wq.T   -> [T, H, G, Dl]
  lk = x @ Wlk.T  -> [T, H, Dl]
  lv = x @ Wlv.T  -> [T, H, Dl]
  scores[h,g,t,s] = (q[t,h,g,:] . lk[s,h,:]) / sqrt(128)
  probs = softmax_s(scores)
  attn[t, h,g,:] = sum_s probs * lv[s,h,:]
  out = attn @ Wo.T
Sharding: 2 heads per core (8 cores); each core computes a full-width
partial of the output projection; partials are summed on the host.

v5 design (t-chunk TC=256, 8 chunks) = v3 structure plus:
 - exp split between ScalarE (Exp activation) and DVE (Schraudolph bf16
   tensor_scalar from f32 PSUM) -- NDVE of every 32 (sb,h) units go to
   DVE. exp is ~286us if left entirely on ScalarE (the v3 bottleneck).
 - merged lk+lv projection (one M=128 proj: lv h0|lv h1|lk h0|lk h1)
   instead of the 4x-replicated lk proj; lk replicated into lkT_rep
   (rows 32g = replica g) by a K=64 PE matmul against R2[h] (identity
   blocks for head h's lk rows, zeros elsewhere) -- saves ~28us of PE.
 - Wo partials DMA'd out in f32 (no bf16 rounding of the output).
Scores keep v3's block-diagonal K=64 form: 2 concurrent row-tile MMs
must write DISJOINT PSUM banks (hardware rule), which forces the
zero-padded qz moving layout (4-way K=32 tiling would need 4 banks).
"""

import numpy as np

T = 2048
C = 2048
HEADS_PER_CORE = 2
DH = 128
DL = 32
G = 4
N_CORES = 8
TC = 256  # t-chunk
PC = 512  # projection chunk
SCALE = 1.0 / np.sqrt(np.float32(DH))
ESCALE = SCALE
EXPA = 184.6635  # bf16 Schraudolph: bits = round(x*EXPA*ESCALE + EXPB)
EXPB = 16250.5
NDVE = 10  # of every 32 (sb,h) exp units go to DVE (Schraudolph)


def build_program(t=T, c=C):
    import concourse.mybir as mybir
    import concourse.tile as tile
    from concourse import bacc
    from concourse.masks import make_identity

    bf16 = mybir.dt.bfloat16
    f32 = mybir.dt.float32

    nc = bacc.Bacc("TRN2", target_bir_lowering=False, debug=False, num_devices=1)

    n_cb = c // 128
    QCOLS = HEADS_PER_CORE * DH  # 256
    KVCOLS = 4 * DL  # 128: lv h0 | lv h1 | lk h0 | lk h1

    xT_d = nc.dram_tensor("xT", [c, t], bf16, kind="ExternalInput").ap()
    wqT_d = nc.dram_tensor("wqT", [c, QCOLS], bf16, kind="ExternalInput").ap()
    wkvT_d = nc.dram_tensor("wkvT", [c, KVCOLS], bf16, kind="ExternalInput").ap()
    woT_d = nc.dram_tensor("woT", [QCOLS, c], bf16, kind="ExternalInput").ap()
    out_d = nc.dram_tensor("out", [t, c], f32, kind="ExternalOutput").ap()

    with tile.TileContext(nc) as tc_:
        _emit(
            nc, tc_, tile, mybir, make_identity, bf16, f32,
            xT_d, wqT_d, wkvT_d, woT_d, out_d,
            t, c, n_cb, QCOLS, KVCOLS,
        )
    nc.compile()
    return nc


def _emit(
    nc, tc_, tile, mybir, make_identity, bf16, f32,
    xT_d, wqT_d, wkvT_d, woT_d, out_d,
    t, c, n_cb, QCOLS, KVCOLS,
):
    H = HEADS_PER_CORE
    n_sb = t // 128  # 16
    n_tc = t // TC  # 8
    EXP = mybir.ActivationFunctionType.Exp
    i16 = mybir.dt.int16
    f32r = mybir.dt.float32r
    MULT = mybir.AluOpType.mult
    ADD = mybir.AluOpType.add
    from contextlib import ExitStack

    ctx = ExitStack()
    with ctx:
        # ---------------- persistent SBUF inputs ----------------
        wpool = ctx.enter_context(tc_.tile_pool(name="wpool", bufs=1))
        xT_sb, wqT_sb, wkvT_sb = [], [], []
        for kb in range(n_cb):
            xT_sb.append(wpool.tile([128, t], bf16, name=f"xT{kb}"))
            wkvT_sb.append(wpool.tile([128, KVCOLS], bf16, name=f"wkvT{kb}"))
            wqT_sb.append(wpool.tile([128, QCOLS], bf16, name=f"wqT{kb}"))
        woT_sb = [wpool.tile([128, c], bf16, name=f"woT{h}") for h in range(H)]
        # DMA order: kv/q weights first, then x in t-quarters (the
        # first-exp chain needs only t 0:512 of every kb).
        for kb in range(n_cb):
            q = nc.sync if kb % 2 == 0 else nc.gpsimd
            q.dma_start(wkvT_sb[kb][:], wkvT_d[kb * 128 : (kb + 1) * 128, :])
            q.dma_start(wqT_sb[kb][:], wqT_d[kb * 128 : (kb + 1) * 128, :])
        for tq in range(4):
            for kb in range(n_cb):
                q = nc.sync if kb % 2 == 0 else nc.gpsimd
                q.dma_start(
                    xT_sb[kb][:, tq * PC : (tq + 1) * PC],
                    xT_d[kb * 128 : (kb + 1) * 128, tq * PC : (tq + 1) * PC],
                )
            if tq == 1:
                for h in range(H):
                    q = nc.sync if h % 2 == 0 else nc.gpsimd
                    q.dma_start(woT_sb[h][:], woT_d[h * 128 : (h + 1) * 128, :])

        ident = wpool.tile([128, 128], bf16, name="ident")
        make_identity(nc, ident[:])
        ones1 = wpool.tile([1, DL], bf16, name="ones1")
        nc.vector.memset(ones1[:], 1.0)

        # ---------------- SBUF working tiles ----------------
        apool = ctx.enter_context(tc_.tile_pool(name="apool", bufs=1))
        # qz: block-diagonal zero-padded q layout. Per 512-col group (one
        # t-chunk, one gp): cols 0:256 hold q of the even group on its own
        # 32-row band, cols 256:512 the odd group; all other rows zero.
        qz = [apool.tile([128, 2 * t], bf16, name=f"qz{h}") for h in range(H)]
        for h in range(H):
            nc.vector.memset(qz[h][:], 0.0)
        # kvT: rows 0:32 lv h0, 32:64 lv h1, 64:96 lk h0, 96:128 lk h1
        # (lv first: the PE transpose needs a base-partition-0 input)
        kvT = apool.tile([128, t], bf16, name="kvT")
        # lkT_rep[h]: rows 32g:32g+32 = replica g of lk head h
        lkT_rep = [apool.tile([128, t], bf16, name=f"lkT_rep{h}") for h in range(H)]
        lv_all = apool.tile([128, 66 * n_sb], bf16, name="lv_all")
        lv_sb = [lv_all[:, 66 * sb : 66 * (sb + 1)] for sb in range(n_sb)]
        # one-shot fill: the ones columns (col 32, 65 of each 66-block)
        # stay 1.0; the lv copies overwrite the rest.
        nc.vector.memset(lv_all[:], 1.0)

        # persistent denominator staging (serialized across chunks by WAR);
        # one [1, 512] tile per (h, gp) -- custom-DVE ops need offset-0 APs
        den = [
            [apool.tile([1, 2 * TC], f32, name=f"den{h}_{gp}") for gp in range(2)]
            for h in range(H)
        ]
        rec = [
            [apool.tile([1, 2 * TC], f32, name=f"rec{h}_{gp}") for gp in range(2)]
            for h in range(H)
        ]
        recb = [
            [apool.tile([1, 2 * TC], bf16, name=f"recb{h}_{gp}") for gp in range(2)]
            for h in range(H)
        ]

        expool = ctx.enter_context(tc_.tile_pool(name="expool", bufs=12))
        qnpool = ctx.enter_context(tc_.tile_pool(name="qnpool", bufs=2))
        atpool = ctx.enter_context(tc_.tile_pool(name="atpool", bufs=4))
        bcspool = ctx.enter_context(tc_.tile_pool(name="bcspool", bufs=3))
        otpool = ctx.enter_context(tc_.tile_pool(name="otpool", bufs=6))

        scpool = ctx.enter_context(tc_.tile_pool(name="scpool", bufs=2, space="PSUM"))
        avpool = ctx.enter_context(tc_.tile_pool(name="avpool", bufs=2, space="PSUM"))
        pjpool = ctx.enter_context(tc_.tile_pool(name="pjpool", bufs=2, space="PSUM"))

        def rep_mm(h, c0, cw):
            """replicate lk head h cols c0:c0+cw into lkT_rep bands via
            SBUF->SBUF DMA (DMA engines are otherwise idle)."""
            for g in range(G):
                q = nc.sync if (g + h) % 2 == 0 else nc.gpsimd
                q.dma_start(
                    lkT_rep[h][32 * g : 32 * (g + 1), c0 : c0 + cw],
                    kvT[64 + 32 * h : 64 + 32 * (h + 1), c0 : c0 + cw],
                )

        # ---------------- emission helpers ----------------
        def qz_scatter(h, src_sb, c0, cw):
            """DMA the natural-layout q [128, cw] at t-offset c0 into the
            block-diagonal qz slots (tci slot index = c0 // TC)."""
            nslot = cw // TC
            for g in range(G):
                dst = qz[h][g * DL : (g + 1) * DL, :].rearrange(
                    "p (s gi c) -> p s gi c", gi=2, c=TC
                )[:, c0 // TC : c0 // TC + nslot, g % 2 : g % 2 + 1, :]
                srcv = src_sb[g * DL : (g + 1) * DL, 0:cw].rearrange(
                    "p (s c) -> p s c", c=TC
                )
                q = nc.sync if (g + h) % 2 == 0 else nc.gpsimd
                q.dma_start(dst, srcv)

        def q_chunk(h, c0, cw, name):
            """q proj chunk -> natural staging tile -> DMA scatter into
            the block-diagonal qz layout."""
            ps = pjpool.tile([128, PC], f32, name=f"ps_{name}", tag="pj")
            for kb in range(n_cb):
                nc.tensor.matmul(
                    ps[:, 0:cw],
                    wqT_sb[kb][:, h * 128 : (h + 1) * 128],
                    xT_sb[kb][:, c0 : c0 + cw],
                    start=(kb == 0),
                    stop=(kb == n_cb - 1),
                )
            qn = qnpool.tile([128, PC], bf16, name=f"qn_{name}", tag="qn")
            nc.vector.tensor_copy(qn[:, 0:cw], ps[:, 0:cw])
            qz_scatter(h, qn, c0, cw)

        def kv_chunk(nch):
            """kv projection chunk (PC cols) + lk replication MMs +
            lv transposes into lv_sb."""
            ps = pjpool.tile([128, PC], f32, name="ps_kv", tag="pj")
            for kb in range(n_cb):
                nc.tensor.matmul(
                    ps[:],
                    wkvT_sb[kb][:],
                    xT_sb[kb][:, nch * PC : (nch + 1) * PC],
                    start=(kb == 0),
                    stop=(kb == n_cb - 1),
                )
            nc.vector.tensor_copy(kvT[:, nch * PC : (nch + 1) * PC], ps[:])
            for h in range(H):
                rep_mm(h, nch * PC, PC)
            for j in range(PC // 128):
                sb = nch * (PC // 128) + j
                pt = pjpool.tile([128, PC], bf16, name="ps_lvT", tag="pj")
                nc.tensor.transpose(
                    pt[:, 0:64],
                    kvT[0:64, sb * 128 : (sb + 1) * 128],
                    ident[0:64, 0:64],
                )
                nc.vector.tensor_copy(lv_sb[sb][:, 0:DL], pt[:, 0:DL])
                nc.vector.tensor_copy(
                    lv_sb[sb][:, DL + 1 : 2 * DL + 1], pt[:, DL : 2 * DL]
                )

        def scores_exp(tci, sb, h, unit):
            """2 block-diag K=64 score MMs (one PSUM bank each) + exp."""
            sc = scpool.tile([128, 4 * TC], f32, name="sc", tag="sc")
            for gp in range(2):
                nc.tensor.matmul(
                    sc[:, gp * 2 * TC : (gp + 1) * 2 * TC],
                    lkT_rep[h][gp * 64 : (gp + 1) * 64, sb * 128 : (sb + 1) * 128],
                    qz[h][gp * 64 : (gp + 1) * 64, tci * 2 * TC : (tci + 1) * 2 * TC],
                    start=True,
                    stop=True,
                    tile_position=(gp * 64, 0),
                )
            ex = expool.tile([128, 4 * TC], bf16, name="ex", tag="ex")
            # DVE exp units: 10 of 32, spread evenly, none in the first
            # two sb iterations (DVE is busy with the previous chunk's
            # norm there -- the avpool rotation waits on it)
            if unit >= 4 and unit % 3 == 1:
                # Schraudolph bf16 exp on DVE (max rel err ~3.3%); high
                # priority: its latency gates the sc-buffer rotation
                with tc_.high_priority():
                    nc.vector.tensor_scalar(
                        ex[:].bitcast(i16), sc[:], float(EXPA * ESCALE), EXPB,
                        MULT, ADD,
                    )
            else:
                nc.scalar.activation(ex[:], sc[:], EXP, scale=float(ESCALE))
            return ex

        def av_accum(sb, av, ex_h):
            """4 AV MMs; av keyed by h, gp on the col-tile axis so both
            gp matmuls become ready together (true pair concurrency)."""
            for h in range(H):
                for gp in range(2):
                    nc.tensor.matmul(
                        av[h][gp * 64 : gp * 64 + DL + 1, :],
                        lv_sb[sb][:, h * (DL + 1) : (h + 1) * (DL + 1)],
                        ex_h[h][:, gp * 2 * TC : (gp + 1) * 2 * TC],
                        start=(sb == 0),
                        stop=(sb == n_sb - 1),
                        skip_group_check=True,
                        tile_position=(0, gp * 64),
                    )

        def norm_a(av):
            """denominator extraction + reciprocal (DVE only). High
            priority: the avpool rotation (next chunk's accumulation)
            waits on the full norm chain."""
            with tc_.high_priority():
                for h in range(H):
                    for gp in range(2):
                        r = gp * 64 + DL
                        nc.vector.tensor_scalar(
                            den[h][gp][:], av[h][r : r + 1, :], 1.0, None, MULT
                        )
                        nc.vector.reciprocal_approx_fast(
                            rec[h][gp][:], den[h][gp][:]
                        )
                        nc.vector.tensor_scalar(
                            recb[h][gp][:], rec[h][gp][:], 1.0, None, MULT
                        )

        def norm_b(av, at, h, pool=None):
            """broadcast + normalize multiply for one head."""
            if pool is None:
                bc = pjpool.tile([128, PC], f32, name="bc", tag="pj")
            else:
                bc = pool.tile([128, 2 * PC], f32, name="bc", tag="sc")[:, 0:PC]
            for gp in range(2):
                nc.tensor.matmul(
                    bc[gp * 64 : gp * 64 + DL, :],
                    ones1[:],
                    recb[h][gp][:],
                    start=True,
                    stop=True,
                    skip_group_check=True,
                    tile_position=(0, gp * 64),
                )
            bcs = bcspool.tile([128, PC], f32, name="bcs", tag="bcs")
            with tc_.high_priority():
                for gp in range(2):
                    nc.vector.tensor_copy(
                        bcs[gp * 64 : gp * 64 + DL, :],
                        bc[gp * 64 : gp * 64 + DL, :],
                    )
                for gp in range(2):
                    for gi in range(2):
                        g = 2 * gp + gi
                        nc.vector.tensor_tensor(
                            at[h][g * DL : (g + 1) * DL, :],
                            av[h][gp * 64 : gp * 64 + DL, gi * TC : (gi + 1) * TC],
                            bcs[gp * 64 : gp * 64 + DL, gi * TC : (gi + 1) * TC],
                            MULT,
                        )

        def wo_chunk(tci, at, ck, pool=None):
            """output chunk ck (of 8): t-block tb, PC out cols oc."""
            tb, oc = divmod(ck, c // PC)
            t0 = tci * TC + tb * 128
            if pool is None:
                wos = pjpool.tile([128, PC], f32, name="wos", tag="pj")
            else:
                wos = pool.tile([128, 2 * PC], f32, name="wos", tag="sc")[:, 0:PC]
            for h in range(H):
                nc.tensor.matmul(
                    wos[:],
                    at[h][:, tb * 128 : (tb + 1) * 128],
                    woT_sb[h][:, oc * PC : (oc + 1) * PC],
                    start=(h == 0),
                    stop=(h == H - 1),
                )
            ot = otpool.tile([128, PC], f32, name="ot", tag="ot")
            if ck % 2 == 0:
                # balance valve: half the PSUM evacuations go to ScalarE
                nc.scalar.copy(ot[:], wos[:])
            else:
                nc.vector.tensor_copy(ot[:], wos[:])
            nc.sync.dma_start(
                out_d[t0 : t0 + 128, oc * PC : (oc + 1) * PC], ot[:]
            )

        def intro_phase1():
            """kb-paced: kv chunk 0 + q chunk 0 (both heads) -- exactly
            the first-exp dependencies, paced by the tq0 xT DMA quarter."""
            pkv = pjpool.tile([128, PC], f32, name="ps_ikv", tag="pj")
            pq = [
                avpool.tile([128, 2 * TC], f32, name=f"ps_iq{h}", tag="av")
                for h in range(H)
            ]
            for kb in range(n_cb):
                nc.tensor.matmul(
                    pkv[:],
                    wkvT_sb[kb][:],
                    xT_sb[kb][:, 0:PC],
                    start=(kb == 0), stop=(kb == n_cb - 1),
                )
                for h in range(H):
                    nc.tensor.matmul(
                        pq[h][:, 0:TC],
                        wqT_sb[kb][:, h * 128 : (h + 1) * 128],
                        xT_sb[kb][:, 0:TC],
                        start=(kb == 0), stop=(kb == n_cb - 1),
                    )
            nc.vector.tensor_copy(kvT[:, 0:PC], pkv[:])
            for h in range(H):
                rep_mm(h, 0, PC)
            for h in range(H):
                qn = qnpool.tile([128, PC], bf16, name=f"qn_i{h}", tag="qn")
                nc.vector.tensor_copy(qn[:, 0:TC], pq[h][:, 0:TC])
                qz_scatter(h, qn, 0, TC)
            for j in range(PC // 128):
                pt = pjpool.tile([128, PC], bf16, name="ps_ilvT", tag="pj")
                nc.tensor.transpose(
                    pt[:, 0:64],
                    kvT[0:64, j * 128 : (j + 1) * 128],
                    ident[0:64, 0:64],
                )
                nc.vector.tensor_copy(lv_sb[j][:, 0:DL], pt[:, 0:DL])
                nc.vector.tensor_copy(
                    lv_sb[j][:, DL + 1 : 2 * DL + 1], pt[:, DL : 2 * DL]
                )

        # ---------------- emission ----------------
        intro_phase1()

        prev = None
        for tci in range(n_tc):
            at = [
                atpool.tile([128, TC], bf16, name=f"at{h}", tag="at")
                for h in range(H)
            ]
            items = []
            if tci == 0:
                items += [
                    lambda: kv_chunk(1),
                    lambda: kv_chunk(2),
                    lambda: kv_chunk(3),
                ]
            else:
                pav, pat, ptci = prev
                items += [
                    lambda: norm_a(pav),
                    lambda: norm_b(pav, pat, 0),
                    lambda: norm_b(pav, pat, 1),
                ]
            if tci % 2 == 0 and tci < n_tc - 1:
                # q chunk covering the next two t-chunks (clamped at the end)
                qw = min(2 * TC, t - (tci + 1) * TC)
                items += [
                    lambda h=h, qw=qw: q_chunk(
                        h, (tci + 1) * TC, qw, f"q{h}"
                    )
                    for h in range(H)
                ]
            if tci > 0:
                items += [lambda k=k: wo_chunk(ptci, pat, k) for k in range(8)]
            # delay av alloc so the in-order PE queue never blocks on the
            # previous chunk's norm (DVE) latency, and emit each
            # av_accum(sb) LAG iterations behind its scores so the PE
            # queue never waits on exp(sb) (the main micro-stall source)
            av_after = 1 if tci == 0 else 4
            av_after = min(av_after, len(items))
            LAG = 2
            av = None
            next_acc = 0
            ex_sb = {}
            emitted = 0
            for sb in range(n_sb):
                for h in range(H):
                    ex_sb[(sb, h)] = scores_exp(tci, sb, h, sb * 2 + h)
                if items:
                    items.pop(0)()
                    emitted += 1
                if av is None and emitted >= av_after:
                    av = [
                        avpool.tile([128, 2 * TC], f32, name=f"av{h}", tag="av")
                        for h in range(H)
                    ]
                if av is not None:
                    while next_acc <= sb - LAG:
                        av_accum(
                            next_acc, av, [ex_sb[(next_acc, h)] for h in range(H)]
                        )
                        next_acc += 1
            while items:
                items.pop(0)()
            while next_acc < n_sb:
                av_accum(next_acc, av, [ex_sb[(next_acc, h)] for h in range(H)])
                next_acc += 1
            prev = (av, at, tci)

        pav, pat, ptci = prev
        norm_a(pav)
        norm_b(pav, pat, 0)
        norm_b(pav, pat, 1, pool=scpool)
        for k in range(8):
            wo_chunk(ptci, pat, k, pool=scpool if k % 2 else None)


# ---------------- host side ----------------


def shard_inputs(x, Wq, Wlk, Wlv, Wo):
    """Returns per-core input dicts (bf16, pre-transposed)."""
    import ml_dtypes

    bf = ml_dtypes.bfloat16
    X = np.ascontiguousarray(x.reshape(-1, x.shape[-1]))  # [T, C]
    xT = np.ascontiguousarray(X.T).astype(bf)
    maps = []
    for core in range(N_CORES):
        h0 = core * HEADS_PER_CORE
        qr = slice(h0 * DH, (h0 + HEADS_PER_CORE) * DH)
        kv_cols = []
        for h in range(HEADS_PER_CORE):
            hr = slice((h0 + h) * DL, (h0 + h + 1) * DL)
            kv_cols.append(Wlv[hr, :].T)  # [C, 32] lv first (rows 0:64)
        for h in range(HEADS_PER_CORE):
            hr = slice((h0 + h) * DL, (h0 + h + 1) * DL)
            kv_cols.append(Wlk[hr, :].T)  # [C, 32] lk rows 64:128
        maps.append(
            {
                "xT": xT,
                "wqT": np.ascontiguousarray(Wq[qr, :].T).astype(bf),
                "wkvT": np.ascontiguousarray(
                    np.concatenate(kv_cols, axis=1)
                ).astype(bf),
                "woT": np.ascontiguousarray(Wo[:, qr].T).astype(bf),
            }
        )
    return maps


_CACHE = {}


def kernel(x, Wq, Wk, Wv, Wlk, Wlv, Wo):
    """Full-input entry point. Wk/Wv are unused by the reference forward."""
    if "nc" not in _CACHE:
        _CACHE["nc"] = build_program()
    nc = _CACHE["nc"]
    from concourse.bass_utils import run_bass_kernel_spmd

    in_maps = shard_inputs(
        np.asarray(x, dtype=np.float32),
        np.asarray(Wq, dtype=np.float32),
        np.asarray(Wlk, dtype=np.float32),
        np.asarray(Wlv, dtype=np.float32),
        np.asarray(Wo, dtype=np.float32),
    )
    res = run_bass_kernel_spmd(nc, in_maps, list(range(N_CORES)))
    out = np.zeros((T, C), dtype=np.float32)
    for r in res.results:
        out += r["out"].astype(np.float32)
    return out.reshape(1, T, C)


def _cache_get():
    return _CACHE["nc"]


# revision 37
# speedup vs baseline: 1.2744x; 1.2744x over previous
"""MLA attention Trainium2 kernel: nn_MultiHeadLatentAttention_31722628448847.

Full computation (B=1, T=2048, C=2048, H=16, G=4, Dl=32):
  q  = x @ Wq.T   -> [T, H, G, Dl]
  lk = x @ Wlk.T  -> [T, H, Dl]
  lv = x @ Wlv.T  -> [T, H, Dl]
  scores[h,g,t,s] = (q[t,h,g,:] . lk[s,h,:]) / sqrt(128)
  probs = softmax_s(scores)
  attn[t, h,g,:] = sum_s probs * lv[s,h,:]
  out = attn @ Wo.T
Sharding: 2 heads per core (8 cores); each core computes a full-width
partial of the output projection; partials are summed on the host.

v5 design (t-chunk TC=256, 8 chunks) = v3 structure plus:
 - exp split between ScalarE (Exp activation) and DVE (Schraudolph bf16
   tensor_scalar from f32 PSUM) -- NDVE of every 32 (sb,h) units go to
   DVE. exp is ~286us if left entirely on ScalarE (the v3 bottleneck).
 - merged lk+lv projection (one M=128 proj: lv h0|lv h1|lk h0|lk h1)
   instead of the 4x-replicated lk proj; lk replicated into lkT_rep
   (rows 32g = replica g) by a K=64 PE matmul against R2[h] (identity
   blocks for head h's lk rows, zeros elsewhere) -- saves ~28us of PE.
 - Wo partials DMA'd out in f32 (no bf16 rounding of the output).
Scores keep v3's block-diagonal K=64 form: 2 concurrent row-tile MMs
must write DISJOINT PSUM banks (hardware rule), which forces the
zero-padded qz moving layout (4-way K=32 tiling would need 4 banks).
"""

import numpy as np

T = 2048
C = 2048
HEADS_PER_CORE = 2
DH = 128
DL = 32
G = 4
N_CORES = 8
TC = 256  # t-chunk
PC = 512  # projection chunk
SCALE = 1.0 / np.sqrt(np.float32(DH))
ESCALE = SCALE
EXPA = 184.6635  # bf16 Schraudolph: bits = round(x*EXPA*ESCALE + EXPB)
EXPB = 16250.5
NDVE = 10  # of every 32 (sb,h) exp units go to DVE (Schraudolph)


def build_program(t=T, c=C):
    import concourse.mybir as mybir
    import concourse.tile as tile
    from concourse import bacc
    from concourse.masks import make_identity

    bf16 = mybir.dt.bfloat16
    f32 = mybir.dt.float32

    nc = bacc.Bacc("TRN2", target_bir_lowering=False, debug=False, num_devices=1)

    n_cb = c // 128
    QCOLS = HEADS_PER_CORE * DH  # 256
    KVCOLS = 4 * DL  # 128: lv h0 | lv h1 | lk h0 | lk h1

    xT_d = nc.dram_tensor("xT", [c, t], bf16, kind="ExternalInput").ap()
    wqT_d = nc.dram_tensor("wqT", [c, QCOLS], bf16, kind="ExternalInput").ap()
    wkvT_d = nc.dram_tensor("wkvT", [c, KVCOLS], bf16, kind="ExternalInput").ap()
    woT_d = nc.dram_tensor("woT", [QCOLS, c], bf16, kind="ExternalInput").ap()
    out_d = nc.dram_tensor("out", [t, c], f32, kind="ExternalOutput").ap()

    with tile.TileContext(nc) as tc_:
        _emit(
            nc, tc_, tile, mybir, make_identity, bf16, f32,
            xT_d, wqT_d, wkvT_d, woT_d, out_d,
            t, c, n_cb, QCOLS, KVCOLS,
        )
    nc.compile()
    return nc


def _emit(
    nc, tc_, tile, mybir, make_identity, bf16, f32,
    xT_d, wqT_d, wkvT_d, woT_d, out_d,
    t, c, n_cb, QCOLS, KVCOLS,
):
    H = HEADS_PER_CORE
    n_sb = t // 128  # 16
    n_tc = t // TC  # 8
    EXP = mybir.ActivationFunctionType.Exp
    i16 = mybir.dt.int16
    f32r = mybir.dt.float32r
    MULT = mybir.AluOpType.mult
    ADD = mybir.AluOpType.add
    from contextlib import ExitStack

    ctx = ExitStack()
    with ctx:
        # ---------------- persistent SBUF inputs ----------------
        wpool = ctx.enter_context(tc_.tile_pool(name="wpool", bufs=1))
        xT_sb, wqT_sb, wkvT_sb = [], [], []
        for kb in range(n_cb):
            xT_sb.append(wpool.tile([128, t], bf16, name=f"xT{kb}"))
            wkvT_sb.append(wpool.tile([128, KVCOLS], bf16, name=f"wkvT{kb}"))
            wqT_sb.append(wpool.tile([128, QCOLS], bf16, name=f"wqT{kb}"))
        woT_sb = [wpool.tile([128, c], bf16, name=f"woT{h}") for h in range(H)]
        # DMA order: per kb, its kv/q weights + x quarter-0 together so
        # the kb-th intro matmul's deps land after ~kb/16 of the first
        # wave (not after ALL weights); then the remaining x quarters.
        for kb in range(n_cb):
            q = nc.sync if kb % 2 == 0 else nc.gpsimd
            q.dma_start(wkvT_sb[kb][:], wkvT_d[kb * 128 : (kb + 1) * 128, :])
            q.dma_start(wqT_sb[kb][:], wqT_d[kb * 128 : (kb + 1) * 128, :])
            q.dma_start(
                xT_sb[kb][:, 0:PC],
                xT_d[kb * 128 : (kb + 1) * 128, 0:PC],
            )
        for tq in range(1, 4):
            for kb in range(n_cb):
                q = nc.sync if kb % 2 == 0 else nc.gpsimd
                q.dma_start(
                    xT_sb[kb][:, tq * PC : (tq + 1) * PC],
                    xT_d[kb * 128 : (kb + 1) * 128, tq * PC : (tq + 1) * PC],
                )
            if tq == 2:
                for h in range(H):
                    q = nc.sync if h % 2 == 0 else nc.gpsimd
                    q.dma_start(woT_sb[h][:], woT_d[h * 128 : (h + 1) * 128, :])

        ident = wpool.tile([128, 128], bf16, name="ident")
        make_identity(nc, ident[:])
        ones1 = wpool.tile([1, DL], bf16, name="ones1")
        nc.vector.memset(ones1[:], 1.0)

        # ---------------- SBUF working tiles ----------------
        apool = ctx.enter_context(tc_.tile_pool(name="apool", bufs=1))
        # qz: block-diagonal zero-padded q layout. Per 512-col group (one
        # t-chunk, one gp): cols 0:256 hold q of the even group on its own
        # 32-row band, cols 256:512 the odd group; all other rows zero.
        qz = [apool.tile([128, 2 * t], bf16, name=f"qz{h}") for h in range(H)]
        for h in range(H):
            nc.vector.memset(qz[h][:], 0.0)
        # kvT: rows 0:32 lv h0, 32:64 lv h1, 64:96 lk h0, 96:128 lk h1
        # (lv first: the PE transpose needs a base-partition-0 input)
        kvT = apool.tile([128, t], bf16, name="kvT")
        # lkT_rep[h]: rows 32g:32g+32 = replica g of lk head h
        lkT_rep = [apool.tile([128, t], bf16, name=f"lkT_rep{h}") for h in range(H)]
        lv_all = apool.tile([128, 66 * n_sb], bf16, name="lv_all")
        lv_sb = [lv_all[:, 66 * sb : 66 * (sb + 1)] for sb in range(n_sb)]
        # one-shot fill: the ones columns (col 32, 65 of each 66-block)
        # stay 1.0; the lv copies overwrite the rest.
        nc.vector.memset(lv_all[:], 1.0)

        # persistent denominator staging (serialized across chunks by WAR);
        # one [1, 512] tile per (h, gp) -- custom-DVE ops need offset-0 APs
        den = [
            [apool.tile([1, 2 * TC], f32, name=f"den{h}_{gp}") for gp in range(2)]
            for h in range(H)
        ]
        rec = [
            [apool.tile([1, 2 * TC], f32, name=f"rec{h}_{gp}") for gp in range(2)]
            for h in range(H)
        ]
        recb = [
            [apool.tile([1, 2 * TC], bf16, name=f"recb{h}_{gp}") for gp in range(2)]
            for h in range(H)
        ]

        expool = ctx.enter_context(tc_.tile_pool(name="expool", bufs=12))
        qnpool = ctx.enter_context(tc_.tile_pool(name="qnpool", bufs=2))
        avspool = ctx.enter_context(tc_.tile_pool(name="avspool", bufs=4))
        atpool = ctx.enter_context(tc_.tile_pool(name="atpool", bufs=4))
        bcspool = ctx.enter_context(tc_.tile_pool(name="bcspool", bufs=3))
        otpool = ctx.enter_context(tc_.tile_pool(name="otpool", bufs=6))

        scpool = ctx.enter_context(tc_.tile_pool(name="scpool", bufs=2, space="PSUM"))
        avpool = ctx.enter_context(tc_.tile_pool(name="avpool", bufs=2, space="PSUM"))
        pjpool = ctx.enter_context(tc_.tile_pool(name="pjpool", bufs=2, space="PSUM"))

        def rep_mm(h, c0, cw):
            """replicate lk head h cols c0:c0+cw into lkT_rep bands via
            SBUF->SBUF DMA (DMA engines are otherwise idle)."""
            for g in range(G):
                q = nc.sync if (g + h) % 2 == 0 else nc.gpsimd
                q.dma_start(
                    lkT_rep[h][32 * g : 32 * (g + 1), c0 : c0 + cw],
                    kvT[64 + 32 * h : 64 + 32 * (h + 1), c0 : c0 + cw],
                )

        # ---------------- emission helpers ----------------
        def qz_scatter(h, src_sb, c0, cw):
            """DMA the natural-layout q [128, cw] at t-offset c0 into the
            block-diagonal qz slots (tci slot index = c0 // TC)."""
            nslot = cw // TC
            for g in range(G):
                dst = qz[h][g * DL : (g + 1) * DL, :].rearrange(
                    "p (s gi c) -> p s gi c", gi=2, c=TC
                )[:, c0 // TC : c0 // TC + nslot, g % 2 : g % 2 + 1, :]
                srcv = src_sb[g * DL : (g + 1) * DL, 0:cw].rearrange(
                    "p (s c) -> p s c", c=TC
                )
                q = nc.sync if (g + h) % 2 == 0 else nc.gpsimd
                q.dma_start(dst, srcv)

        def q_chunk(h, c0, cw, name):
            """q proj chunk -> natural staging tile -> DMA scatter into
            the block-diagonal qz layout."""
            ps = pjpool.tile([128, PC], f32, name=f"ps_{name}", tag="pj")
            for kb in range(n_cb):
                nc.tensor.matmul(
                    ps[:, 0:cw],
                    wqT_sb[kb][:, h * 128 : (h + 1) * 128],
                    xT_sb[kb][:, c0 : c0 + cw],
                    start=(kb == 0),
                    stop=(kb == n_cb - 1),
                )
            qn = qnpool.tile([128, PC], bf16, name=f"qn_{name}", tag="qn")
            nc.vector.tensor_copy(qn[:, 0:cw], ps[:, 0:cw])
            qz_scatter(h, qn, c0, cw)

        def kv_chunk(nch):
            """kv projection chunk (PC cols) + lk replication MMs +
            lv transposes into lv_sb."""
            ps = pjpool.tile([128, PC], f32, name="ps_kv", tag="pj")
            for kb in range(n_cb):
                nc.tensor.matmul(
                    ps[:],
                    wkvT_sb[kb][:],
                    xT_sb[kb][:, nch * PC : (nch + 1) * PC],
                    start=(kb == 0),
                    stop=(kb == n_cb - 1),
                )
            nc.vector.tensor_copy(kvT[:, nch * PC : (nch + 1) * PC], ps[:])
            for h in range(H):
                rep_mm(h, nch * PC, PC)
            for j in range(PC // 128):
                sb = nch * (PC // 128) + j
                pt = pjpool.tile([128, PC], bf16, name="ps_lvT", tag="pj")
                nc.tensor.transpose(
                    pt[:, 0:64],
                    kvT[0:64, sb * 128 : (sb + 1) * 128],
                    ident[0:64, 0:64],
                )
                nc.vector.tensor_copy(lv_sb[sb][:, 0:DL], pt[:, 0:DL])
                nc.vector.tensor_copy(
                    lv_sb[sb][:, DL + 1 : 2 * DL + 1], pt[:, DL : 2 * DL]
                )

        def scores_exp(tci, sb, h, unit):
            """2 block-diag K=64 score MMs (one PSUM bank each) + exp."""
            sc = scpool.tile([128, 4 * TC], f32, name="sc", tag="sc")
            for gp in range(2):
                nc.tensor.matmul(
                    sc[:, gp * 2 * TC : (gp + 1) * 2 * TC],
                    lkT_rep[h][gp * 64 : (gp + 1) * 64, sb * 128 : (sb + 1) * 128],
                    qz[h][gp * 64 : (gp + 1) * 64, tci * 2 * TC : (tci + 1) * 2 * TC],
                    start=True,
                    stop=True,
                    tile_position=(gp * 64, 0),
                )
            ex = expool.tile([128, 4 * TC], bf16, name="ex", tag="ex")
            # DVE exp units: 10 of 32, spread evenly, none in the first
            # two sb iterations (DVE is busy with the previous chunk's
            # norm there -- the avpool rotation waits on it)
            if unit >= 4 and unit % 3 == 1:
                # Schraudolph bf16 exp on DVE (max rel err ~3.3%); high
                # priority: its latency gates the sc-buffer rotation
                with tc_.high_priority():
                    nc.vector.tensor_scalar(
                        ex[:].bitcast(i16), sc[:], float(EXPA * ESCALE), EXPB,
                        MULT, ADD,
                    )
            else:
                nc.scalar.activation(ex[:], sc[:], EXP, scale=float(ESCALE))
            return ex

        def av_accum(sb, av, ex_h):
            """4 AV MMs; av keyed by h, gp on the col-tile axis so both
            gp matmuls become ready together (true pair concurrency)."""
            for h in range(H):
                for gp in range(2):
                    nc.tensor.matmul(
                        av[h][gp * 64 : gp * 64 + DL + 1, :],
                        lv_sb[sb][:, h * (DL + 1) : (h + 1) * (DL + 1)],
                        ex_h[h][:, gp * 2 * TC : (gp + 1) * 2 * TC],
                        start=(sb == 0),
                        stop=(sb == n_sb - 1),
                        skip_group_check=True,
                        tile_position=(0, gp * 64),
                    )

        def avs_copy(av, avs):
            """evacuate av PSUM -> SBUF on ScalarE (has slack) so the
            av banks free for the next chunk's accumulation ~5us sooner
            than waiting out the whole DVE norm chain."""
            for h in range(H):
                nc.scalar.copy(avs[h][:], av[h][:])

        def norm_a(avs):
            """denominator extraction + reciprocal (DVE, from SBUF)."""
            for h in range(H):
                for gp in range(2):
                    r = gp * 64 + DL
                    nc.vector.tensor_scalar(
                        den[h][gp][:], avs[h][r : r + 1, :], 1.0, None, MULT
                    )
                    nc.vector.reciprocal_approx_fast(rec[h][gp][:], den[h][gp][:])
                    nc.vector.tensor_scalar(
                        recb[h][gp][:], rec[h][gp][:], 1.0, None, MULT
                    )

        def norm_b(av, at, h, pool=None):
            """broadcast + normalize multiply for one head."""
            if pool is None:
                bc = pjpool.tile([128, PC], f32, name="bc", tag="pj")
            else:
                bc = pool.tile([128, 2 * PC], f32, name="bc", tag="sc")[:, 0:PC]
            for gp in range(2):
                nc.tensor.matmul(
                    bc[gp * 64 : gp * 64 + DL, :],
                    ones1[:],
                    recb[h][gp][:],
                    start=True,
                    stop=True,
                    skip_group_check=True,
                    tile_position=(0, gp * 64),
                )
            bcs = bcspool.tile([128, PC], f32, name="bcs", tag="bcs")
            for gp in range(2):
                nc.vector.tensor_copy(
                    bcs[gp * 64 : gp * 64 + DL, :], bc[gp * 64 : gp * 64 + DL, :]
                )
            for gp in range(2):
                for gi in range(2):
                    g = 2 * gp + gi
                    nc.vector.tensor_tensor(
                        at[h][g * DL : (g + 1) * DL, :],
                        av[h][gp * 64 : gp * 64 + DL, gi * TC : (gi + 1) * TC],
                        bcs[gp * 64 : gp * 64 + DL, gi * TC : (gi + 1) * TC],
                        MULT,
                    )

        def wo_chunk(tci, at, ck, pool=None):
            """output chunk ck (of 8): t-block tb, PC out cols oc."""
            tb, oc = divmod(ck, c // PC)
            t0 = tci * TC + tb * 128
            if pool is None:
                wos = pjpool.tile([128, PC], f32, name="wos", tag="pj")
            else:
                wos = pool.tile([128, 2 * PC], f32, name="wos", tag="sc")[:, 0:PC]
            for h in range(H):
                nc.tensor.matmul(
                    wos[:],
                    at[h][:, tb * 128 : (tb + 1) * 128],
                    woT_sb[h][:, oc * PC : (oc + 1) * PC],
                    start=(h == 0),
                    stop=(h == H - 1),
                )
            ot = otpool.tile([128, PC], f32, name="ot", tag="ot")
            if ck % 2 == 0:
                # balance valve: half the PSUM evacuations go to ScalarE
                nc.scalar.copy(ot[:], wos[:])
            else:
                nc.vector.tensor_copy(ot[:], wos[:])
            nc.sync.dma_start(
                out_d[t0 : t0 + 128, oc * PC : (oc + 1) * PC], ot[:]
            )

        def intro_phase1():
            """kb-paced: kv chunk 0 + q chunk 0 (both heads) -- exactly
            the first-exp dependencies, paced by the tq0 xT DMA quarter."""
            pkv = pjpool.tile([128, PC], f32, name="ps_ikv", tag="pj")
            pq = [
                avpool.tile([128, 2 * TC], f32, name=f"ps_iq{h}", tag="av")
                for h in range(H)
            ]
            for kb in range(n_cb):
                nc.tensor.matmul(
                    pkv[:],
                    wkvT_sb[kb][:],
                    xT_sb[kb][:, 0:PC],
                    start=(kb == 0), stop=(kb == n_cb - 1),
                )
                for h in range(H):
                    nc.tensor.matmul(
                        pq[h][:, 0:TC],
                        wqT_sb[kb][:, h * 128 : (h + 1) * 128],
                        xT_sb[kb][:, 0:TC],
                        start=(kb == 0), stop=(kb == n_cb - 1),
                    )
            nc.vector.tensor_copy(kvT[:, 0:PC], pkv[:])
            for h in range(H):
                rep_mm(h, 0, PC)
            for h in range(H):
                qn = qnpool.tile([128, PC], bf16, name=f"qn_i{h}", tag="qn")
                nc.vector.tensor_copy(qn[:, 0:TC], pq[h][:, 0:TC])
                qz_scatter(h, qn, 0, TC)
            for j in range(PC // 128):
                pt = pjpool.tile([128, PC], bf16, name="ps_ilvT", tag="pj")
                nc.tensor.transpose(
                    pt[:, 0:64],
                    kvT[0:64, j * 128 : (j + 1) * 128],
                    ident[0:64, 0:64],
                )
                nc.vector.tensor_copy(lv_sb[j][:, 0:DL], pt[:, 0:DL])
                nc.vector.tensor_copy(
                    lv_sb[j][:, DL + 1 : 2 * DL + 1], pt[:, DL : 2 * DL]
                )

        # ---------------- emission ----------------
        intro_phase1()

        prev = None
        for tci in range(n_tc):
            at = [
                atpool.tile([128, TC], bf16, name=f"at{h}", tag="at")
                for h in range(H)
            ]
            items = []
            if tci == 0:
                items += [
                    lambda: kv_chunk(1),
                    lambda: kv_chunk(2),
                    lambda: kv_chunk(3),
                ]
            else:
                pav, pat, ptci = prev
                pavs = [
                    avspool.tile([128, 2 * TC], f32, name=f"avs{h}", tag="avs")
                    for h in range(H)
                ]
                items += [
                    lambda: avs_copy(pav, pavs),
                    lambda: norm_a(pavs),
                    lambda: norm_b(pavs, pat, 0),
                    lambda: norm_b(pavs, pat, 1),
                ]
            if tci % 2 == 0 and tci < n_tc - 1:
                # q chunk covering the next two t-chunks (clamped at the end)
                qw = min(2 * TC, t - (tci + 1) * TC)
                items += [
                    lambda h=h, qw=qw: q_chunk(
                        h, (tci + 1) * TC, qw, f"q{h}"
                    )
                    for h in range(H)
                ]
            if tci > 0:
                items += [lambda k=k: wo_chunk(ptci, pat, k) for k in range(8)]
            # delay av alloc so the in-order PE queue never blocks on the
            # previous chunk's norm (DVE) latency, and emit each
            # av_accum(sb) LAG iterations behind its scores so the PE
            # queue never waits on exp(sb) (the main micro-stall source)
            av_after = 1 if tci == 0 else 2
            av_after = min(av_after, len(items))
            LAG = 2
            av = None
            next_acc = 0
            ex_sb = {}
            emitted = 0
            for sb in range(n_sb):
                for h in range(H):
                    ex_sb[(sb, h)] = scores_exp(tci, sb, h, sb * 2 + h)
                if items:
                    items.pop(0)()
                    emitted += 1
                if av is None and emitted >= av_after:
                    av = [
                        avpool.tile([128, 2 * TC], f32, name=f"av{h}", tag="av")
                        for h in range(H)
                    ]
                if av is not None:
                    while next_acc <= sb - LAG:
                        av_accum(
                            next_acc, av, [ex_sb[(next_acc, h)] for h in range(H)]
                        )
                        next_acc += 1
            while items:
                items.pop(0)()
            while next_acc < n_sb:
                av_accum(next_acc, av, [ex_sb[(next_acc, h)] for h in range(H)])
                next_acc += 1
            prev = (av, at, tci)

        pav, pat, ptci = prev
        pavs = [
            avspool.tile([128, 2 * TC], f32, name=f"avs_f{h}", tag="avs")
            for h in range(H)
        ]
        avs_copy(pav, pavs)
        norm_a(pavs)
        norm_b(pavs, pat, 0)
        norm_b(pavs, pat, 1, pool=scpool)
        for k in range(8):
            wo_chunk(ptci, pat, k, pool=scpool if k % 2 else None)


# ---------------- host side ----------------


def shard_inputs(x, Wq, Wlk, Wlv, Wo):
    """Returns per-core input dicts (bf16, pre-transposed)."""
    import ml_dtypes

    bf = ml_dtypes.bfloat16
    X = np.ascontiguousarray(x.reshape(-1, x.shape[-1]))  # [T, C]
    xT = np.ascontiguousarray(X.T).astype(bf)
    maps = []
    for core in range(N_CORES):
        h0 = core * HEADS_PER_CORE
        qr = slice(h0 * DH, (h0 + HEADS_PER_CORE) * DH)
        kv_cols = []
        for h in range(HEADS_PER_CORE):
            hr = slice((h0 + h) * DL, (h0 + h + 1) * DL)
            kv_cols.append(Wlv[hr, :].T)  # [C, 32] lv first (rows 0:64)
        for h in range(HEADS_PER_CORE):
            hr = slice((h0 + h) * DL, (h0 + h + 1) * DL)
            kv_cols.append(Wlk[hr, :].T)  # [C, 32] lk rows 64:128
        maps.append(
            {
                "xT": xT,
                "wqT": np.ascontiguousarray(Wq[qr, :].T).astype(bf),
                "wkvT": np.ascontiguousarray(
                    np.concatenate(kv_cols, axis=1)
                ).astype(bf),
                "woT": np.ascontiguousarray(Wo[:, qr].T).astype(bf),
            }
        )
    return maps


_CACHE = {}


def kernel(x, Wq, Wk, Wv, Wlk, Wlv, Wo):
    """Full-input entry point. Wk/Wv are unused by the reference forward."""
    if "nc" not in _CACHE:
        _CACHE["nc"] = build_program()
    nc = _CACHE["nc"]
    from concourse.bass_utils import run_bass_kernel_spmd

    in_maps = shard_inputs(
        np.asarray(x, dtype=np.float32),
        np.asarray(Wq, dtype=np.float32),
        np.asarray(Wlk, dtype=np.float32),
        np.asarray(Wlv, dtype=np.float32),
        np.asarray(Wo, dtype=np.float32),
    )
    res = run_bass_kernel_spmd(nc, in_maps, list(range(N_CORES)))
    out = np.zeros((T, C), dtype=np.float32)
    for r in res.results:
        out += r["out"].astype(np.float32)
    return out.reshape(1, T, C)


def _cache_get():
    return _CACHE["nc"]
